# revision 56
# baseline (speedup 1.0000x reference)
"""Trainium2 Bass kernel for MultiHeadSelfAttention (B=4, L=2048, H=1024, NH=16).

Sharding: 8 cores = 4 batches x 2 head-groups (8 heads each).

Per-core design (v2):
- Projections: 8x128 contraction (no bias row); q/k biases added by DVE
  during the PSUM->SBUF copy (per-partition bias AP); v bias folded into
  the output bias on the host (attn rows sum to 1).
- Attention loop is query-chunk (512) outer, head-pair inner. Per key-tile j:
  the two heads of a pair run ROW-TILED CONCURRENT QK^T matmuls (K=64 on
  array rows 0-63 / 64-127), one fused exp over both heads' scores
  [128,1024] on ScalarE, mask multiplies split DVE/Pool, and two AV
  matmuls accumulating numerator+denominator (ones-columns trick).
- Pair projections for pair p+1 and the output projection for each query
  chunk are interleaved under the ACT-bound attention steady state.
- Output written bf16; host sums the two head-group partials in f32.
"""

import os
import sys

os.environ.setdefault("JAX_PLATFORMS", "")
try:
    import concourse.bass as bass  # noqa: F401
except ImportError:
    sys.path.insert(0, "/opt/trn_rl_repo")

import numpy as np
import ml_dtypes

import concourse.bass as bass
import concourse.mybir as mybir
import concourse.tile as tile
from concourse import bacc
from concourse import bass_utils

BF16 = mybir.dt.bfloat16
F32 = mybir.dt.float32

USE_TILE_POSITION = True
USE_POOL_MASK = True
USE_POOL_MEMSET = True
DEBUG_DUMP = False

B, L, H = 4, 2048, 1024
NH, HD = 16, 64
NCORES = 8
HPC = NH // 2          # heads per core = 8
CPC = H // 2           # channels per core = 512
KC = 8                 # contraction chunks of 128 (no bias row)
PAIRS = HPC // 2       # 4 head pairs per core
LT = L // 128          # 16 key tiles
QCN = 4                # query chunks of 512


def build_nc(repeat=1):
    nc = bacc.Bacc("TRN2", target_bir_lowering=False, debug=False,
                   num_devices=NCORES)

    xT = nc.dram_tensor("xT", [H, L], BF16, kind="ExternalInput").ap()
    wqT = nc.dram_tensor("wqT", [H, CPC], BF16, kind="ExternalInput").ap()
    wkT = nc.dram_tensor("wkT", [H, CPC], BF16, kind="ExternalInput").ap()
    wvT = nc.dram_tensor("wvT", [H, CPC], BF16, kind="ExternalInput").ap()
    woT = nc.dram_tensor("woT", [CPC, H], BF16, kind="ExternalInput").ap()
    maskT = nc.dram_tensor("maskT", [L, L], BF16, kind="ExternalInput").ap()
    bqkT = nc.dram_tensor("bqkT", [128, 2 * PAIRS], F32,
                          kind="ExternalInput").ap()
    out = nc.dram_tensor("out", [L, H], BF16, kind="ExternalOutput").ap()
    dbg = None
    if DEBUG_DUMP:
        dbg = {nm: nc.dram_tensor(f"dbg_{nm}", shp, BF16,
                                  kind="ExternalOutput").ap()
               for nm, shp in (("qT", [128, PAIRS, L]), ("kT", [128, PAIRS, L]),
                               ("v", [128, LT, HPC * 128]),
                               ("aoT", [128, PAIRS, L]))}
        for nm in ("s0", "pm0", "nd0", "rec0"):
            dbg[nm] = nc.dram_tensor(f"dbg_{nm}", [128, 1024], F32,
                                     kind="ExternalOutput").ap()

    with tile.TileContext(nc) as tc:
        for _ in range(repeat):
            mhsa_body(tc, xT, wqT, wkT, wvT, woT, maskT, bqkT, out, dbg)
    nc.compile()
    return nc


def mhsa_body(tc, xT, wqT, wkT, wvT, woT, maskT, bqkT, out, dbg=None):
    nc = tc.nc
    Exp = mybir.ActivationFunctionType.Exp
    mult = mybir.AluOpType.mult

    xT_r = xT.rearrange("(kc p) t -> p kc t", p=128)
    wq_r = wqT.rearrange("(kc p) c -> p kc c", p=128)
    wk_r = wkT.rearrange("(kc p) c -> p kc c", p=128)
    wv_r = wvT.rearrange("(kc p) c -> p kc c", p=128)
    wo_r = woT.rearrange("(kc p) c -> p kc c", p=128)
    mask_r = maskT.rearrange("(jt p) q -> p jt q", p=128)
    out_r = out.rearrange("(tt p) c -> p tt c", p=128)

    import contextlib
    ctx = contextlib.ExitStack()
    with ctx:
        wpool = ctx.enter_context(tc.tile_pool(name="weights", bufs=1))
        xpool = ctx.enter_context(tc.tile_pool(name="xpool", bufs=1))
        qkv_sb = ctx.enter_context(tc.tile_pool(name="qkv_sb", bufs=1))
        ao_pool = ctx.enter_context(tc.tile_pool(name="ao", bufs=1))
        mpool = ctx.enter_context(tc.tile_pool(name="mask", bufs=2))
        pmpool = ctx.enter_context(tc.tile_pool(name="pm", bufs=4))
        opool = ctx.enter_context(tc.tile_pool(name="osb", bufs=2))
        rpool = ctx.enter_context(tc.tile_pool(name="rec", bufs=2))
        mgpool = ctx.enter_context(tc.tile_pool(name="mg", bufs=2))
        s_ps = ctx.enter_context(tc.tile_pool(name="s_ps", bufs=2,
                                              space="PSUM"))
        nd_ps = ctx.enter_context(tc.tile_pool(name="nd_ps", bufs=1,
                                               space="PSUM"))
        po_ps = ctx.enter_context(tc.tile_pool(name="po_ps", bufs=2,
                                               space="PSUM"))

        wq_sb = wpool.tile([128, KC, CPC], BF16, tag="wq")
        wk_sb = wpool.tile([128, KC, CPC], BF16, tag="wk")
        wv_sb = wpool.tile([128, KC, CPC], BF16, tag="wv")
        wo_sb = wpool.tile([128, PAIRS, H], BF16, tag="wo")
        bqk_sb = wpool.tile([128, 2 * PAIRS], F32, tag="bqk")
        x_sb = xpool.tile([128, KC, L], BF16, tag="x")

        qT_sb = qkv_sb.tile([128, PAIRS, L], BF16, tag="qT")
        kT_sb = qkv_sb.tile([128, PAIRS, L], BF16, tag="kT")
        v_sb = qkv_sb.tile([128, LT, HPC * 128], BF16, tag="v")
        aoT_sb = ao_pool.tile([128, PAIRS, L], BF16, tag="aoT")

        # DMA priority: q/k weights + x first (pair-0 projection is the
        # critical path), then v weights, then the rest
        nc.sync.dma_start(bqk_sb[:], bqkT)
        # x loaded in partition halves: the K=64 chain-A matmuls (rows 0-63)
        # can start before the upper halves arrive
        for kc in range(KC):
            nc.sync.dma_start(wq_sb[:, kc, :], wq_r[:, kc, :])
            nc.sync.dma_start(wk_sb[:, kc, :], wk_r[:, kc, :])
            nc.sync.dma_start(x_sb[0:64, kc, :], xT_r[0:64, kc, :])
        for kc in range(KC):
            nc.sync.dma_start(x_sb[64:128, kc, :], xT_r[64:128, kc, :])
        for kc in range(KC):
            nc.sync.dma_start(wv_sb[:, kc, :], wv_r[:, kc, :])
        nc.sync.dma_start(wo_sb[:], wo_r)

        # mask chunks are reloaded per (pair, qc); prefetch the first one
        mseq = [(p, q) for p in range(PAIRS) for q in range(QCN)]
        mtiles = {}

        def fetch_mask(p, q):
            mtiles[(p, q)] = mpool.tile([128, LT, 512], BF16, tag="mask",
                                        name=f"m{p}_{q}")
            nc.sync.dma_start(mtiles[(p, q)][:],
                              mask_r[:, :, q * 512:(q + 1) * 512])

        fetch_mask(*mseq[0])

        # v layout per (t, head): [ones 64 | v 64] so one AV matmul yields
        # denominator (psum rows 0-63) and numerator (rows 64-127); the
        # denominator must land on partitions 0-63 because the custom DVE
        # reciprocal reads its input at the OUTPUT's base partition
        v_aug = v_sb[:].rearrange("p t (h two d) -> p t h two d", two=2, d=64)
        (nc.gpsimd if USE_POOL_MEMSET else nc.vector).memset(
            v_aug[:, :, :, 0, :], 1.0)

        # Projection/out-projection contractions are split into two K=64
        # chains on alternating PE row groups so every LDWEIGHTS overlaps the
        # other group's in-flight matmul (HW: ~3x). Each chain owns its own
        # psum bank (A=rows 0-63 @ rg0, B=rows 64-127 @ rg1) -- the proven
        # QK row-tiling pattern, no shared-bank chains. Merge = ACT copy of
        # B to SBUF (single-psum-input, verifier-legal) + DVE add.
        add_op = mybir.AluOpType.add

        def split_chain(psA, psB, lhs_rhs, n):
            for i in range(n):
                lhsT, rhs = lhs_rhs(i)
                for hh in range(2):
                    rb = hh * 64
                    nc.tensor.matmul(
                        (psA if hh == 0 else psB)[:],
                        lhsT[rb:rb + 64, :],
                        rhs[rb:rb + 64, :],
                        start=(i == 0), stop=(i == n - 1),
                        tile_position=(rb, 0),
                    )

        mergecnt = [0]

        def merge_ab(dst, psA, psB, bias=None, reshaped=False):
            mg = mgpool.tile([128, 512], F32, tag="mg")
            # alternate the B-bank copy between ACT and DVE to balance load
            if mergecnt[0] % 2 == 0:
                nc.scalar.copy(mg[:], psB[:])
            else:
                nc.vector.tensor_copy(mg[:], psB[:])
            mergecnt[0] += 1
            if reshaped:
                nc.vector.tensor_tensor(
                    dst,
                    psA[:].rearrange("p (h d) -> p h d", d=64),
                    mg[:].rearrange("p (h d) -> p h d", d=64),
                    add_op)
            else:
                nc.vector.tensor_tensor(dst, psA[:], mg[:], add_op)
            if bias is not None:
                nc.vector.tensor_scalar_add(dst, dst, bias)

        def proj_qk_chunk(pair, tk, which):
            # one 512-token chunk of the q^T or k^T projection for a pair
            w_sb, dst, bi = ((wq_sb, qT_sb, 2 * pair) if which == 0 else
                             (wk_sb, kT_sb, 2 * pair + 1))
            psA = po_ps.tile([128, 512], F32, tag="po",
                             name=f"qkA{pair}_{tk}_{which}")
            psB = po_ps.tile([128, 512], F32, tag="po",
                             name=f"qkB{pair}_{tk}_{which}")
            split_chain(psA, psB, lambda kc: (
                w_sb[:, kc, pair * 128:(pair + 1) * 128],
                x_sb[:, kc, tk * 512:(tk + 1) * 512]), KC)
            merge_ab(dst[:, pair, tk * 512:(tk + 1) * 512], psA, psB,
                     bias=bqk_sb[:, bi:bi + 1])

        def proj_v(t):
            psA = po_ps.tile([128, 512], F32, tag="po", name=f"vA{t}")
            psB = po_ps.tile([128, 512], F32, tag="po", name=f"vB{t}")
            split_chain(psA, psB, lambda kc: (
                x_sb[:, kc, t * 128:(t + 1) * 128], wv_sb[:, kc, :]), KC)
            merge_ab(v_aug[:, t, :, 1, :], psA, psB, reshaped=True)

        def outproj_chain(t, half):
            # half of one token tile of the output projection
            psA = po_ps.tile([128, 512], F32, tag="po", name=f"oA{t}_{half}")
            psB = po_ps.tile([128, 512], F32, tag="po", name=f"oB{t}_{half}")
            split_chain(psA, psB, lambda pr: (
                aoT_sb[:, pr, t * 128:(t + 1) * 128],
                wo_sb[:, pr, half * 512:(half + 1) * 512]), PAIRS)
            o_sb = osb_tiles[t]
            merge_ab(o_sb[:, half * 512:(half + 1) * 512], psA, psB)
            if half == 1:
                nc.sync.dma_start(out_r[:, t, :], o_sb[:])

        osb_tiles = {}

        # minimal prefix: only q0[tk0] (covers qc0's queries) and all of k0;
        # q0[tk1-3] are background work inside pair0/qc0 (needed at qc1)
        proj_qk_chunk(0, 0, 0)
        for tk in range(4):
            proj_qk_chunk(0, tk, 1)

        # ---------------- attention with interleaved background work --------
        mcnt = 0
        carry = []      # prev block's final AV + recip/mults, emitted after
                        # the next block's first QK so the boundary chain
                        # (mask15 -> AV15) hides under the next block's exp(0)
        for pair in range(PAIRS):
            h1 = 2 * pair
            for qc in range(QCN):
                seq_i = pair * QCN + qc
                if seq_i + 1 < len(mseq):
                    fetch_mask(*mseq[seq_i + 1])
                mt = mtiles.pop((pair, qc))
                qs = slice(qc * 512, (qc + 1) * 512)

                # background PE work to drain inside this (pair, qc) loop
                bg = []
                if pair == 0 and qc == 0:
                    bg = [(proj_qk_chunk, (0, tk, 0)) for tk in (1, 2, 3)]
                if pair + 1 < PAIRS:
                    if qc == 1:
                        bg = [(proj_qk_chunk, (pair + 1, tk, 0))
                              for tk in range(4)]
                    elif qc == 2:
                        bg = [(proj_qk_chunk, (pair + 1, tk, 1))
                              for tk in range(4)]
                if pair == PAIRS - 1 and qc > 0:
                    prev = qc - 1
                    for tt in range(4):
                        t = prev * 4 + tt
                        osb_tiles[t] = opool.tile([128, H], BF16, tag="osb",
                                                  name=f"o{t}")
                        bg.append((outproj_chain, (t, 0)))
                        bg.append((outproj_chain, (t, 1)))
                bg_per_j = (len(bg) + LT - 1) // LT if bg else 0

                nd = nd_ps.tile([128, 1024], F32, tag="nd",
                                name=f"nd{pair}_{qc}")

                def emit_av(j, pm, nd=nd, h1=h1):
                    # nd/h1 bound as defaults: emit_av may be invoked from
                    # the NEXT block via the carry, after nd/h1 are rebound
                    for hh in range(2):
                        nc.tensor.matmul(
                            nd[:, hh * 512:(hh + 1) * 512],
                            v_sb[:, j, (h1 + hh) * 128:(h1 + hh + 1) * 128],
                            pm[:, hh * 512:(hh + 1) * 512],
                            start=(j == 0), stop=(j == LT - 1),
                        )

                # software-pipelined: AV(j-1) is emitted after QK(j) so the
                # in-order PE never waits on the exp->mask chain of the
                # current iteration
                pend = None
                for j in range(LT):
                    if pair == 0 and qc == 0:
                        proj_v(j)
                    s = s_ps.tile([128, 1024], F32, tag="s",
                                  name=f"s{pair}_{qc}_{j}")
                    for hh in range(2):
                        rb = hh * 64
                        nc.tensor.matmul(
                            s[:, hh * 512:(hh + 1) * 512],
                            kT_sb[rb:rb + 64, pair, j * 128:(j + 1) * 128],
                            qT_sb[rb:rb + 64, pair, qs],
                            start=True, stop=True,
                            **({"tile_position": (rb, 0)}
                               if USE_TILE_POSITION else {}),
                        )
                    if j == 0:
                        for th in carry:
                            th()
                        carry = []
                    if pend is not None:
                        emit_av(*pend)
                    pm = pmpool.tile([128, 1024], BF16, tag="pm")
                    nc.scalar.activation(pm[:], s[:], Exp, scale=0.125)
                    for hh in range(2):
                        # last iteration's masks stay on the fast DVE: they
                        # gate AV(15) -> next block's QK on the in-order PE
                        eng = nc.gpsimd if USE_POOL_MASK and j < LT - 1 and (
                            (mcnt % 4 == 1 and hh == 1) or
                            (mcnt % 4 == 2 and hh == 0) or
                            (mcnt % 4 == 3 and hh == 0)) else nc.vector
                        eng.tensor_tensor(
                            pm[:, hh * 512:(hh + 1) * 512],
                            pm[:, hh * 512:(hh + 1) * 512],
                            mt[:, j, :], mult)
                    mcnt += 1
                    pend = (j, pm)
                    for _ in range(bg_per_j):
                        if bg:
                            fn, args = bg.pop(0)
                            fn(*args)
                def finalize(pend=pend, emit_av=emit_av, nd=nd, pair=pair,
                             qc=qc, qs=qs):
                    emit_av(*pend)
                    rec = rpool.tile([64, 1024], F32, tag="rec",
                                     name=f"rec{pair}_{qc}")
                    nc.vector.reciprocal_approx_fast(rec[:], nd[0:64, :])
                    for hh in range(2):
                        nc.vector.tensor_tensor(
                            aoT_sb[hh * 64:(hh + 1) * 64, pair, qs],
                            nd[64:128, hh * 512:(hh + 1) * 512],
                            rec[:, hh * 512:(hh + 1) * 512],
                            mult)
                carry = [finalize]

        for th in carry:
            th()
        carry = []

        # final query chunk's output projection (nothing left to hide it under)
        for tt in range(4):
            t = (QCN - 1) * 4 + tt
            osb_tiles[t] = opool.tile([128, H], BF16, tag="osb", name=f"o{t}")
            outproj_chain(t, 0)
            outproj_chain(t, 1)

        if dbg is not None:
            nc.sync.dma_start(dbg["qT"], qT_sb[:])
            nc.sync.dma_start(dbg["kT"], kT_sb[:])
            nc.sync.dma_start(dbg["v"], v_sb[:])
            nc.sync.dma_start(dbg["aoT"], aoT_sb[:])


_NC_CACHE = None


def get_nc():
    global _NC_CACHE
    if _NC_CACHE is None:
        _NC_CACHE = build_nc()
    return _NC_CACHE


def make_in_maps(x, attn_mask, Wq, bq, Wk, bk, Wv, bv, Wo, bo):
    bf = ml_dtypes.bfloat16
    x = np.asarray(x, np.float32)
    attn_mask = np.asarray(attn_mask)
    in_maps = []
    for core in range(NCORES):
        b, g = divmod(core, 2)
        cs = slice(g * CPC, (g + 1) * CPC)
        m = {"xT": np.ascontiguousarray(x[b].T).astype(bf)}
        for name, W in (("wqT", Wq), ("wkT", Wk), ("wvT", Wv)):
            m[name] = np.ascontiguousarray(
                np.asarray(W, np.float32)[cs, :].T).astype(bf)
        m["woT"] = np.ascontiguousarray(
            np.asarray(Wo, np.float32)[:, cs].T).astype(bf)
        m["maskT"] = np.ascontiguousarray(attn_mask[b, 0].T).astype(bf)
        bqk = np.zeros((128, 2 * PAIRS), np.float32)
        bq_s = np.asarray(bq, np.float32)[cs]
        bk_s = np.asarray(bk, np.float32)[cs]
        for p in range(PAIRS):
            bqk[:, 2 * p] = bq_s[p * 128:(p + 1) * 128]
            bqk[:, 2 * p + 1] = bk_s[p * 128:(p + 1) * 128]
        m["bqkT"] = bqk
        in_maps.append(m)
    return in_maps


def gather(results, Wo, bv, bo):
    add = (np.asarray(bo, np.float32)
           + np.asarray(bv, np.float32) @ np.asarray(Wo, np.float32).T)
    out = np.empty((B, L, H), np.float32)
    for b in range(B):
        out[b] = (results[2 * b]["out"].astype(np.float32)
                  + results[2 * b + 1]["out"].astype(np.float32) + add)
    return out


def kernel(x, attn_mask, Wq, bq, Wk, bk, Wv, bv, Wo, bo):
    nc = get_nc()
    in_maps = make_in_maps(x, attn_mask, Wq, bq, Wk, bk, Wv, bv, Wo, bo)
    res = bass_utils.run_bass_kernel_spmd(nc, in_maps,
                                          core_ids=list(range(NCORES)))
    return gather(res.results, Wo, bv, bo)
